# revision 38
# baseline (speedup 1.0000x reference)
"""DMP network kernel for Trainium2 (8 NeuronCores, pure data parallel).

Math: the reference is a 54->54 linear layer followed by a 301-step Euler
integration of a DMP (dynamic movement primitive). The phase variable xp and
hence the RBF activations psi are batch-independent, and the (y, z) scan is a
linear time-invariant recurrence driven by fx = (g - y0) * (w @ P_t). The
whole scan collapses to the closed form

    Y[b, d, t] = a_t * y0 + beta_t * g + (g - y0) * (w @ Q)[b, d, t]

with a, beta [T] and Q [N, T] computed on the host from c / sigma2 in float64.

Scaling a batch row of x by a per-row scalar commutes through any matmul, so
(g - y0) * (w @ Q) = (x_ext * dcol) @ (W2w.T @ Q) with x_ext = [x, 1] and
dcol = g - y0.

Device pipeline (per 128-row batch tile; x arrives host-transposed AND
host-duplicated as fp16 [121, batch]: rows 0..53 x, 54..56 ones, 57..63 zero,
64..117 x again, 118..120 ones):

  1. HBC matmul per 4-tile subgroup: hb [128, 512] = ch.T @ xin[0:55], where
     ch's columns replicate the dcol coefficient across partitions 0..54 (and
     64..118) and put the y0/g coefficients at partitions 55,56 / 119,120.
  2. One VectorE multiply per subgroup: mt [121, 512] = xin * hb.
  3. Per tile, two row-tiled CONCURRENT matmuls (d0 on PE rows 0..56, d1 on
     rows 64..120) into one 2-bank PSUM tile [128, 1024]:
       ps[:,   0:304] = mt[0:57].T  @ cy[0:57]     (DOF 0, full Y incl a/beta)
       ps[:, 512:816] = mt[64:121].T @ cy[64:121]  (DOF 1)
  4. ONE fused strided copy [128, 2, 304] PSUM -> fp16 SBUF (ScalarE/VectorE
     rotation ~9:7, interleaved so neither engine serializes a group).
  5. Group-major fp16 output DMA on the sync ring: per partition one
     contiguous 64B-aligned chunk per 8-tile group (group 0 leaves in 2-tile
     quarters to start the stream early; the last group in 4+2+2 pieces to
     shorten the tail). The host undoes the [g, p, n, f] -> [b, f] shuffle
     and upcasts to f32.

Scheduling notes (from perfetto traces): the first ~6us after the body start
move almost no data (~150 B/ns aggregate), so the ramp-critical loads are
split across both HWDGE rings and everything else is held back -- the
1024-col x chunks ride the scalar ring FIFO *behind* x0a/x0b, one chunk per
group, consumed just-in-time. A short fp32 warm-up matmul burst bridges the
PE from the preamble to the first real matmul so the HAM clock gate reaches
8/8 (fp16 matmuls also avoid the f32r cold-clock 2-cyc/col penalty). The
fp16 I/O keeps rel err ~4e-4 (the harness gate is 2e-2) and halves the HBM
traffic, which is the roofline resource: ~12 MB/core at ~350 GB/s.
"""

import numpy as np

# -- problem constants (fixed by the reference) -------------------------------
N = 25
DOF = 2
TAU = 3.0
DT = 0.01
A_X = 2.0
A_Z = 48.0
B_Z = A_Z / 4.0
T = 301
D_IN = 54           # DOF * (N + 2)
B = 65536
N_CORES = 8
B_CORE = B // N_CORES          # 8192
P = 128                        # batch rows per tile
N_TILES = B_CORE // P          # 64
X_CHUNK = 8                    # tiles per input DMA
G_TILES = 8                    # tiles per output group (DMA granularity)
S_TILES = 4                    # tiles per hb/mult subgroup
N_GROUPS = N_TILES // G_TILES  # 8
D_PAD = 55                     # 54 features + ones row
T_PAD = 304                    # padded tile row: 64B-aligned output chunks
W_HI = 64                      # partition offset of the DOF-1 block
ROWS = 121                     # input image rows (57 used + 7 pad + 57 used)
YROW = DOF * T                 # 602
D1_OFF = 512                   # DOF-1 column offset inside the 2-bank psum


# -- host-side closed-form constants ------------------------------------------
def _closed_form_consts(c, sigma2):
    """a [T], beta [T], Q [N, T] in float64."""
    c = np.asarray(c, np.float64)
    sigma2 = np.asarray(sigma2, np.float64)
    alpha = DT / TAU

    xp = np.empty(T)
    xp[0] = 1.0
    for t in range(T - 1):
        xp[t + 1] = xp[t] - (A_X * xp[t] / TAU) * DT
    psi = np.exp(-0.5 * (xp[:, None] - c[None, :]) ** 2 / sigma2[None, :])  # [T, N]
    S = psi.sum(1)
    Pmat = (psi * (xp / S)[:, None]).T                                      # [N, T]

    A = np.array([[1.0, alpha], [-alpha * A_Z * B_Z, 1.0 - alpha * A_Z]])
    a = np.empty(T)
    bvec = np.empty(T)
    M = np.eye(2)
    for t in range(T):
        a[t] = M[0, 0]
        bvec[t] = M[0, 1]
        M = A @ M
    beta = A_Z * B_Z * alpha * np.concatenate([[0.0], np.cumsum(bvec)[:-1]])

    H = np.zeros((T, T))
    for t in range(1, T):
        H[:t, t] = alpha * bvec[t - 1::-1]
    Q = Pmat @ H                                                            # [N, T]
    return a, beta, Q


def _host_inputs(x, W, b, c, sigma2, scale):
    """Build per-core input maps (numpy)."""
    a, beta, Q = _closed_form_consts(c, sigma2)

    W2 = np.asarray(W, np.float64) * np.asarray(scale, np.float64)[:, None]
    b2 = np.asarray(b, np.float64) * np.asarray(scale, np.float64)

    # w2e[:, j] = 55-vector [W2[j, :], b2[j]] -- the ones row carries the bias
    w2e = np.concatenate([W2.T, b2[None, :]], axis=0)       # [55, 54]

    # head-broadcast coefficients ch [55, 128] fp16
    ch = np.zeros((D_PAD, P), np.float64)
    # Y-matmul coefficients cy [121, 302] f32: rows 0..56 d0, 64..120 d1
    cy = np.zeros((ROWS, T_PAD), np.float64)
    for d, lo in ((0, 0), (1, W_HI)):
        base = d * (N + 2)
        dc = w2e[:, base + 1] - w2e[:, base]
        ch[:, lo:lo + D_PAD] = dc[:, None]
        ch[:, lo + D_PAD] = w2e[:, base]          # y0_d coeff
        ch[:, lo + D_PAD + 1] = w2e[:, base + 1]  # g_d coeff
        cy[lo:lo + D_PAD, 0:T] = w2e[:, base + 2:base + 2 + N] @ Q
        cy[lo + D_PAD, 0:T] = a
        cy[lo + D_PAD + 1, 0:T] = beta
    # merged const image: cols 0..303 = cy, cols 304..431 rows 0..54 = ch
    cc = np.zeros((ROWS, T_PAD + P), np.float16)
    cc[:, 0:T_PAD] = cy.astype(np.float16)
    cc[0:D_PAD, T_PAD:T_PAD + P] = ch.astype(np.float16)

    # host-transposed + duplicated fp16 x image [121, B]
    xT = np.zeros((ROWS, B), np.float16)
    xf = np.asarray(x, np.float32).T.astype(np.float16)       # [54, B]
    xT[0:D_IN] = xf
    xT[D_IN:D_PAD + 2] = 1.0                                  # rows 54,55,56
    xT[W_HI:W_HI + D_IN] = xf
    xT[W_HI + D_IN:ROWS] = 1.0                                # rows 118,119,120

    in_maps = []
    for ci in range(N_CORES):
        in_maps.append({
            "x": np.ascontiguousarray(xT[:, ci * B_CORE:(ci + 1) * B_CORE]),
            "cc": cc,
        })
    return in_maps


# -- bass program --------------------------------------------------------------
_NC_CACHE = None


def _build_program():
    global _NC_CACHE
    if _NC_CACHE is not None:
        return _NC_CACHE

    import concourse.bacc as bacc
    import concourse.tile as tile
    from concourse import mybir
    from contextlib import ExitStack

    f16 = mybir.dt.float16
    f32 = mybir.dt.float32
    f32r = mybir.dt.float32r

    nc = bacc.Bacc(
        "TRN2",
        target_bir_lowering=False,
        debug=False,
        num_devices=N_CORES,
    )
    x_d = nc.declare_dram_parameter("x", [ROWS, B_CORE], f16, isOutput=False)
    cc_d = nc.declare_dram_parameter("cc", [ROWS, T_PAD + P], f16,
                                     isOutput=False)
    y_d = nc.declare_dram_parameter("y", [N_GROUPS, P, G_TILES * DOF * T_PAD],
                                    f16, isOutput=True)

    CW = X_CHUNK * P               # 1024 input columns per chunk
    SW = S_TILES * P               # 512 columns per hb/mult subgroup

    with tile.TileContext(nc) as tc, ExitStack() as ctx:
        consts = ctx.enter_context(tc.tile_pool(name="consts", bufs=1))
        xin_p = ctx.enter_context(tc.tile_pool(name="xin", bufs=4))
        x0_p = ctx.enter_context(tc.tile_pool(name="x0", bufs=2))
        mt_p = ctx.enter_context(tc.tile_pool(name="mt", bufs=3))
        yout_p = ctx.enter_context(tc.tile_pool(name="yout", bufs=3))
        wu_p = ctx.enter_context(tc.tile_pool(name="wu", bufs=1))
        hb_p = ctx.enter_context(tc.tile_pool(name="hb", bufs=2, space="PSUM"))
        ps_p = ctx.enter_context(tc.tile_pool(name="ps", bufs=3, space="PSUM"))

        # startup: the early-DMA window is slow and latency-limited per ring,
        # so split the ramp-critical loads across both HWDGE rings: consts on
        # sync, the two first-chunk halves on scalar. Nothing else competes.
        cc_sb = consts.tile([ROWS, T_PAD + P], f16)
        nc.sync.dma_start(cc_sb[:], cc_d[:])
        cy_sb = cc_sb[:, 0:T_PAD]
        ch_sb = cc_sb[0:D_PAD, T_PAD:T_PAD + P]
        x0a = x0_p.tile([ROWS, SW], f16)
        nc.scalar.dma_start(x0a[:], x_d[:, 0:SW])
        x0b = x0_p.tile([ROWS, SW], f16)
        nc.scalar.dma_start(x0b[:], x_d[:, SW:CW])

        # PE warm-up: back-to-back dummy matmuls on a memset tile bridge the
        # PE from the preamble to the first hb matmul so the HAM clock gate
        # reaches 8/8 before the real stream begins.
        wt = wu_p.tile([P, 256], f32)
        nc.vector.memset(wt[:], 0.0)
        ps_w = ps_p.tile([P, 2 * D1_OFF], f32, tag="ps")
        for _ in range(3):
            nc.tensor.matmul(ps_w[:, 0:256], wt[:, 0:P], wt[:],
                             start=True, stop=True)

        # x chunks on the scalar ring, FIFO behind x0a/x0b so they cannot
        # flood the slow early-DMA window (~160 B/ns until ~15us). One
        # 1024-col chunk per group, consumed just-in-time: chunk k arrives
        # ~1.5us after chunk k-1, groups consume one every ~2.5-4us.
        def ct_dma(k, w):
            t = xin_p.tile([ROWS, w], f16, name=f"xc{k}")
            c0 = (k + 1) * CW
            nc.scalar.dma_start(t[:], x_d[:, c0:c0 + w])
            return t

        ct = [ct_dma(0, CW), ct_dma(1, CW), ct_dma(2, CW),
              ct_dma(3, 4 * CW)]

        for gi in range(N_GROUPS):

            ysb = yout_p.tile([P, G_TILES, DOF, T_PAD], f16)
            for s in range(G_TILES // S_TILES):
                si = gi * (G_TILES // S_TILES) + s
                if gi == 0:
                    xv = (x0a, x0b)[s]
                    sc = 0
                elif gi < 4:
                    xv = ct[gi - 1]
                    sc = s * SW
                else:
                    xv = ct[3]
                    sc = (gi - 4) * CW + s * SW

                hb = hb_p.tile([P, SW], f32)
                nc.tensor.matmul(hb[:], ch_sb[:], xv[0:D_PAD, sc:sc + SW],
                                 start=True, stop=True)

                # mt rows: [x*dcol0 (55); y0_0; g_0; 0 x7; x*dcol1; y0_1; g_1]
                mt = mt_p.tile([ROWS, SW], f16, tag="mt")
                nc.vector.tensor_mul(mt[:], xv[0:ROWS, sc:sc + SW],
                                     hb[0:ROWS, :])

                for j in range(S_TILES):
                    jc = j * P
                    jj = s * S_TILES + j          # tile index inside group
                    i = si * S_TILES + j          # global tile index

                    ps = ps_p.tile([P, 2 * D1_OFF], f32, tag="ps")
                    nc.tensor.matmul(ps[:, 0:T_PAD],
                                     mt[0:D_PAD + 2, jc:jc + P],
                                     cy_sb[0:D_PAD + 2, :],
                                     start=True, stop=True)
                    nc.tensor.matmul(ps[:, D1_OFF:D1_OFF + T_PAD],
                                     mt[W_HI:ROWS, jc:jc + P],
                                     cy_sb[W_HI:ROWS, :],
                                     start=True, stop=True)

                    src = ps.rearrange("p (b f) -> p b f", b=2)[:, :, 0:T_PAD]
                    dst = ysb[:, jj]
                    # ~9:7 ScalarE:VectorE rotation (ScalarE also issues the
                    # x-chunk dispatches, DVE runs the mults), interleaved so
                    # neither engine serializes a whole group.
                    if i % 16 in (1, 3, 5, 7, 13, 15):
                        nc.vector.tensor_copy(dst, src)
                    else:
                        nc.scalar.copy(dst, src)

            # output dispatch: group 0 streams out in 2-tile quarters (the
            # stream starts ~3us earlier), the last group in 4+2+2 pieces
            # (short tail after the final copy), the rest as full groups.
            TROW = DOF * T_PAD
            if gi <= 1:
                cuts = (0, 2, 4, 6, 8)
            elif gi == 2:
                cuts = (0, 4, 8)
            elif gi == N_GROUPS - 1:
                cuts = (0, 4, 6, 8)
            else:
                cuts = (0, 8)
            for lo, hi in zip(cuts[:-1], cuts[1:]):
                piece = ysb[:, lo:hi]
                nc.sync.dma_start(y_d[gi][:, lo * TROW:hi * TROW],
                                  piece.rearrange("p g b f -> p (g b f)"))

    nc.compile()
    _NC_CACHE = nc
    return nc


_LAST_RESULTS = None


def kernel(x, W, b, c, sigma2, scale):
    global _LAST_RESULTS
    from concourse.bass_utils import run_bass_kernel_spmd

    assert x.shape == (B, D_IN), x.shape
    nc = _build_program()
    in_maps = _host_inputs(x, W, b, c, sigma2, scale)
    res = run_bass_kernel_spmd(nc, in_maps, list(range(N_CORES)))
    _LAST_RESULTS = res
    outs = []
    for ci in range(N_CORES):
        yc = np.asarray(res.results[ci]["y"])            # [8, 128, 4864] fp16
        yc = yc.reshape(N_GROUPS, P, G_TILES, DOF, T_PAD)[..., :T]
        yc = yc.transpose(0, 2, 1, 3, 4).reshape(B_CORE, YROW)
        outs.append(yc)
    return np.concatenate(outs, axis=0).astype(np.float32)


# revision 39
# speedup vs baseline: 1.0520x; 1.0520x over previous
"""DMP network kernel for Trainium2 (8 NeuronCores, pure data parallel).

Math: the reference is a 54->54 linear layer followed by a 301-step Euler
integration of a DMP (dynamic movement primitive). The phase variable xp and
hence the RBF activations psi are batch-independent, and the (y, z) scan is a
linear time-invariant recurrence driven by fx = (g - y0) * (w @ P_t). The
whole scan collapses to the closed form

    Y[b, d, t] = a_t * y0 + beta_t * g + (g - y0) * (w @ Q)[b, d, t]

with a, beta [T] and Q [N, T] computed on the host from c / sigma2 in float64.

Scaling a batch row of x by a per-row scalar commutes through any matmul, so
(g - y0) * (w @ Q) = (x_ext * dcol) @ (W2w.T @ Q) with x_ext = [x, 1] and
dcol = g - y0.

Device pipeline (per 128-row batch tile; x arrives host-transposed AND
host-duplicated as fp16 [121, batch]: rows 0..53 x, 54..56 ones, 57..63 zero,
64..117 x again, 118..120 ones):

  1. HBC matmul per 4-tile group: hb [128, 512] = ch.T @ xin[0:55], where ch's
     columns replicate the dcol coefficient across partitions 0..54 (and
     64..118) and put the y0/g coefficients at partitions 55,56 / 119,120.
  2. One VectorE multiply per group: mt [121, 512] = xin * hb.
  3. Per tile, two row-tiled CONCURRENT matmuls (d0 on PE rows 0..56, d1 on
     rows 64..120) into one 2-bank PSUM tile [128, 1024]:
       ps[:,   0:302] = mt[0:57].T  @ cy[0:57]     (DOF 0, full Y incl a/beta)
       ps[:, 512:814] = mt[64:121].T @ cy[64:121]  (DOF 1)
  4. ONE fused strided copy [128, 2, 301] PSUM -> fp16 SBUF (ScalarE/VectorE
     rotation 5:3).
  5. Group-major fp16 output DMA: per partition one contiguous 4816 B chunk;
     the host undoes the [g, p, n, f] -> [b, f] shuffle and upcasts to f32.

The fp16 I/O keeps rel err ~6e-4 (the harness gate is 2e-2) and halves the
HBM traffic, which is the roofline resource: ~11 MB/core at ~350 GB/s.
"""

import numpy as np

# -- problem constants (fixed by the reference) -------------------------------
N = 25
DOF = 2
TAU = 3.0
DT = 0.01
A_X = 2.0
A_Z = 48.0
B_Z = A_Z / 4.0
T = 301
D_IN = 54           # DOF * (N + 2)
B = 65536
N_CORES = 8
B_CORE = B // N_CORES          # 8192
P = 128                        # batch rows per tile
N_TILES = B_CORE // P          # 64
X_CHUNK = 8                    # tiles per input DMA
G_TILES = 8                    # tiles per output group (DMA granularity)
S_TILES = 4                    # tiles per hb/mult subgroup
N_GROUPS = N_TILES // G_TILES  # 8
D_PAD = 55                     # 54 features + ones row
T_PAD = 304                    # padded tile row: 64B-aligned output chunks
W_HI = 64                      # partition offset of the DOF-1 block
ROWS = 121                     # input image rows (57 used + 7 pad + 57 used)
YROW = DOF * T                 # 602
D1_OFF = 512                   # DOF-1 column offset inside the 2-bank psum


# -- host-side closed-form constants ------------------------------------------
def _closed_form_consts(c, sigma2):
    """a [T], beta [T], Q [N, T] in float64."""
    c = np.asarray(c, np.float64)
    sigma2 = np.asarray(sigma2, np.float64)
    alpha = DT / TAU

    xp = np.empty(T)
    xp[0] = 1.0
    for t in range(T - 1):
        xp[t + 1] = xp[t] - (A_X * xp[t] / TAU) * DT
    psi = np.exp(-0.5 * (xp[:, None] - c[None, :]) ** 2 / sigma2[None, :])  # [T, N]
    S = psi.sum(1)
    Pmat = (psi * (xp / S)[:, None]).T                                      # [N, T]

    A = np.array([[1.0, alpha], [-alpha * A_Z * B_Z, 1.0 - alpha * A_Z]])
    a = np.empty(T)
    bvec = np.empty(T)
    M = np.eye(2)
    for t in range(T):
        a[t] = M[0, 0]
        bvec[t] = M[0, 1]
        M = A @ M
    beta = A_Z * B_Z * alpha * np.concatenate([[0.0], np.cumsum(bvec)[:-1]])

    H = np.zeros((T, T))
    for t in range(1, T):
        H[:t, t] = alpha * bvec[t - 1::-1]
    Q = Pmat @ H                                                            # [N, T]
    return a, beta, Q


def _host_inputs(x, W, b, c, sigma2, scale):
    """Build per-core input maps (numpy)."""
    a, beta, Q = _closed_form_consts(c, sigma2)

    W2 = np.asarray(W, np.float64) * np.asarray(scale, np.float64)[:, None]
    b2 = np.asarray(b, np.float64) * np.asarray(scale, np.float64)

    # w2e[:, j] = 55-vector [W2[j, :], b2[j]] -- the ones row carries the bias
    w2e = np.concatenate([W2.T, b2[None, :]], axis=0)       # [55, 54]

    # head-broadcast coefficients ch [55, 128] fp16
    ch = np.zeros((D_PAD, P), np.float64)
    # Y-matmul coefficients cy [121, 302] f32: rows 0..56 d0, 64..120 d1
    cy = np.zeros((ROWS, T_PAD), np.float64)
    for d, lo in ((0, 0), (1, W_HI)):
        base = d * (N + 2)
        dc = w2e[:, base + 1] - w2e[:, base]
        ch[:, lo:lo + D_PAD] = dc[:, None]
        ch[:, lo + D_PAD] = w2e[:, base]          # y0_d coeff
        ch[:, lo + D_PAD + 1] = w2e[:, base + 1]  # g_d coeff
        cy[lo:lo + D_PAD, 0:T] = w2e[:, base + 2:base + 2 + N] @ Q
        cy[lo + D_PAD, 0:T] = a
        cy[lo + D_PAD + 1, 0:T] = beta
    # merged const image: cols 0..303 = cy, cols 304..431 rows 0..54 = ch
    cc = np.zeros((ROWS, T_PAD + P), np.float16)
    cc[:, 0:T_PAD] = cy.astype(np.float16)
    cc[0:D_PAD, T_PAD:T_PAD + P] = ch.astype(np.float16)

    # host-transposed + duplicated fp16 x image [121, B]
    xT = np.zeros((ROWS, B), np.float16)
    xf = np.asarray(x, np.float32).T.astype(np.float16)       # [54, B]
    xT[0:D_IN] = xf
    xT[D_IN:D_PAD + 2] = 1.0                                  # rows 54,55,56
    xT[W_HI:W_HI + D_IN] = xf
    xT[W_HI + D_IN:ROWS] = 1.0                                # rows 118,119,120

    in_maps = []
    for ci in range(N_CORES):
        in_maps.append({
            "x": np.ascontiguousarray(xT[:, ci * B_CORE:(ci + 1) * B_CORE]),
            "cc": cc,
        })
    return in_maps


# -- bass program --------------------------------------------------------------
_NC_CACHE = None


def _build_program():
    global _NC_CACHE
    if _NC_CACHE is not None:
        return _NC_CACHE

    import concourse.bacc as bacc
    import concourse.tile as tile
    from concourse import mybir
    from contextlib import ExitStack

    f16 = mybir.dt.float16
    f32 = mybir.dt.float32
    f32r = mybir.dt.float32r

    nc = bacc.Bacc(
        "TRN2",
        target_bir_lowering=False,
        debug=False,
        num_devices=N_CORES,
    )
    x_d = nc.declare_dram_parameter("x", [ROWS, B_CORE], f16, isOutput=False)
    cc_d = nc.declare_dram_parameter("cc", [ROWS, T_PAD + P], f16,
                                     isOutput=False)
    y_d = nc.declare_dram_parameter("y", [N_GROUPS, P, G_TILES * DOF * T_PAD],
                                    f16, isOutput=True)

    CW = X_CHUNK * P               # 1024 input columns per chunk
    SW = S_TILES * P               # 512 columns per hb/mult subgroup

    with tile.TileContext(nc) as tc, ExitStack() as ctx:
        consts = ctx.enter_context(tc.tile_pool(name="consts", bufs=1))
        xin_p = ctx.enter_context(tc.tile_pool(name="xin", bufs=3))
        x0_p = ctx.enter_context(tc.tile_pool(name="x0", bufs=2))
        mt_p = ctx.enter_context(tc.tile_pool(name="mt", bufs=3))
        yout_p = ctx.enter_context(tc.tile_pool(name="yout", bufs=3))
        wu_p = ctx.enter_context(tc.tile_pool(name="wu", bufs=1))
        hb_p = ctx.enter_context(tc.tile_pool(name="hb", bufs=2, space="PSUM"))
        ps_p = ctx.enter_context(tc.tile_pool(name="ps", bufs=3, space="PSUM"))

        # startup: the early-DMA window is slow and latency-limited per ring,
        # so split the ramp-critical loads across both HWDGE rings: consts on
        # sync, the two first-chunk halves on scalar. Nothing else competes.
        cc_sb = consts.tile([ROWS, T_PAD + P], f16)
        nc.sync.dma_start(cc_sb[:], cc_d[:])
        cy_sb = cc_sb[:, 0:T_PAD]
        ch_sb = cc_sb[0:D_PAD, T_PAD:T_PAD + P]
        x0a = x0_p.tile([ROWS, SW], f16)
        nc.scalar.dma_start(x0a[:], x_d[:, 0:SW])
        x0b = x0_p.tile([ROWS, SW], f16)
        nc.scalar.dma_start(x0b[:], x_d[:, SW:CW])

        # PE warm-up: back-to-back dummy matmuls on a memset tile bridge the
        # PE from the preamble to the first hb matmul so the HAM clock gate
        # reaches 8/8 before the real stream begins.
        wt = wu_p.tile([P, 256], f32)
        nc.vector.memset(wt[:], 0.0)
        ps_w = ps_p.tile([P, 2 * D1_OFF], f32, tag="ps")
        for _ in range(3):
            nc.tensor.matmul(ps_w[:, 0:256], wt[:, 0:P], wt[:],
                             start=True, stop=True)

        # x chunks on the scalar ring, FIFO behind x0a/x0b so they cannot
        # flood the slow early-DMA window (~160 B/ns until ~15us). One
        # 1024-col chunk per group, consumed just-in-time: chunk k arrives
        # ~1.5us after chunk k-1, groups consume one every ~2.5-4us.
        def ct_dma(k):
            t = xin_p.tile([ROWS, CW], f16, name=f"xc{k}")
            c0 = (k + 1) * CW
            nc.scalar.dma_start(t[:], x_d[:, c0:c0 + CW])
            return t

        ct = [None] * (N_GROUPS - 1)
        ct[0] = ct_dma(0)
        ct[1] = ct_dma(1)

        for gi in range(N_GROUPS):

            ysb = yout_p.tile([P, G_TILES, DOF, T_PAD], f16)
            for s in range(G_TILES // S_TILES):
                si = gi * (G_TILES // S_TILES) + s
                if gi == 0:
                    xv = (x0a, x0b)[s]
                    sc = 0
                else:
                    xv = ct[gi - 1]
                    sc = s * SW

                hb = hb_p.tile([P, SW], f32)
                nc.tensor.matmul(hb[:], ch_sb[:], xv[0:D_PAD, sc:sc + SW],
                                 start=True, stop=True)

                # mt rows: [x*dcol0 (55); y0_0; g_0; 0 x7; x*dcol1; y0_1; g_1]
                mt = mt_p.tile([ROWS, SW], f16, tag="mt")
                nc.vector.tensor_mul(mt[:], xv[0:ROWS, sc:sc + SW],
                                     hb[0:ROWS, :])

                for j in range(S_TILES):
                    jc = j * P
                    jj = s * S_TILES + j          # tile index inside group
                    i = si * S_TILES + j          # global tile index

                    ps = ps_p.tile([P, 2 * D1_OFF], f32, tag="ps")
                    nc.tensor.matmul(ps[:, 0:T_PAD],
                                     mt[0:D_PAD + 2, jc:jc + P],
                                     cy_sb[0:D_PAD + 2, :],
                                     start=True, stop=True)
                    nc.tensor.matmul(ps[:, D1_OFF:D1_OFF + T_PAD],
                                     mt[W_HI:ROWS, jc:jc + P],
                                     cy_sb[W_HI:ROWS, :],
                                     start=True, stop=True)

                    src = ps.rearrange("p (b f) -> p b f", b=2)[:, :, 0:T_PAD]
                    dst = ysb[:, jj]
                    # ~9:7 ScalarE:VectorE rotation (ScalarE also issues the
                    # x-chunk dispatches, DVE runs the mults), interleaved so
                    # neither engine serializes a whole group.
                    if i % 16 in (1, 3, 5, 7, 10, 13, 15):
                        nc.vector.tensor_copy(dst, src)
                    else:
                        nc.scalar.copy(dst, src)

            # output dispatch: group 0 streams out in 2-tile quarters (the
            # stream starts ~3us earlier), the last group in 4+2+2 pieces
            # (short tail after the final copy), the rest as full groups.
            TROW = DOF * T_PAD
            if gi == 0:
                cuts = (0, 2, 4, 6, 8)
            elif gi == N_GROUPS - 1:
                cuts = (0, 4, 6, 8)
            else:
                cuts = (0, 8)
            for lo, hi in zip(cuts[:-1], cuts[1:]):
                piece = ysb[:, lo:hi]
                nc.sync.dma_start(y_d[gi][:, lo * TROW:hi * TROW],
                                  piece.rearrange("p g b f -> p (g b f)"))
            if gi + 2 < N_GROUPS - 1:
                ct[gi + 2] = ct_dma(gi + 2)

    nc.compile()
    _NC_CACHE = nc
    return nc


_LAST_RESULTS = None


def kernel(x, W, b, c, sigma2, scale):
    global _LAST_RESULTS
    from concourse.bass_utils import run_bass_kernel_spmd

    assert x.shape == (B, D_IN), x.shape
    nc = _build_program()
    in_maps = _host_inputs(x, W, b, c, sigma2, scale)
    res = run_bass_kernel_spmd(nc, in_maps, list(range(N_CORES)))
    _LAST_RESULTS = res
    outs = []
    for ci in range(N_CORES):
        yc = np.asarray(res.results[ci]["y"])            # [8, 128, 4864] fp16
        yc = yc.reshape(N_GROUPS, P, G_TILES, DOF, T_PAD)[..., :T]
        yc = yc.transpose(0, 2, 1, 3, 4).reshape(B_CORE, YROW)
        outs.append(yc)
    return np.concatenate(outs, axis=0).astype(np.float32)
